# revision 5
# baseline (speedup 1.0000x reference)
"""AbsoluteLearnedPE kernel v7: host-side transpose/add/bf16-cast,
device is a pure bf16 matmul stream; bf16 output upconverted on host.

Per core (data-parallel over batch): logits = q_b @ E^T + E @ (k_b+E)^T.
Host feeds stripe-major bf16 tensors [KS, 128, DTILES, 512] so each
(tensor, stripe) is one contiguous-row DMA (8KB per partition line);
stripe 0 is split per-d for a fast first-group unlock. Inputs ride the
Sync/Scalar/GpSimd rings in parallel; outputs (bf16, half the bytes)
ride the GpSimd software-DGE ring only, so they never queue behind the
bulk input descriptors and the end-of-kernel drain is short.
"""

import numpy as np

B, Q, K, D = 8, 2048, 2048, 1024
DTILES = D // 128     # 8
QT = Q // 128         # 16
KSTRIPE = 512
KS = K // KSTRIPE     # 4

_CACHE = {}
TRACE = False


def _build():
    from concourse import bacc
    import concourse.mybir as mybir
    import concourse.tile as tile

    f32 = mybir.dt.float32
    bf16 = mybir.dt.bfloat16

    nc = bacc.Bacc("TRN2", target_bir_lowering=False, debug=False, num_devices=B)
    qTs = nc.dram_tensor("qTs", [KS, 128, DTILES, KSTRIPE], bf16,
                         kind="ExternalInput").ap()
    eTs = nc.dram_tensor("eTs", [KS, 128, DTILES, KSTRIPE], bf16,
                         kind="ExternalInput").ap()
    kpeTs = nc.dram_tensor("kpeTs", [KS, 128, DTILES, KSTRIPE], bf16,
                           kind="ExternalInput").ap()
    out16 = nc.dram_tensor("out16", [Q, K], bf16, kind="ExternalOutput").ap()

    with tile.TileContext(nc) as tc:
        with tc.tile_pool(name="big", bufs=1) as big, \
             tc.tile_pool(name="outp", bufs=8) as outp, \
             tc.tile_pool(name="mps", bufs=8, space="PSUM") as mps:

            # SBUF layout [128, KS, DTILES, 512] bf16 (32KB/partition each).
            q_sb = big.tile([128, KS, DTILES, KSTRIPE], bf16, tag="qT")
            e_sb = big.tile([128, KS, DTILES, KSTRIPE], bf16, tag="eT")
            kpe_sb = big.tile([128, KS, DTILES, KSTRIPE], bf16, tag="kpeT")

            # Stripe 0 per-d chunks (fast unlock of the first matmul group),
            # spread across the three DMA-capable queues.
            for d in range(DTILES):
                nc.sync.dma_start(out=e_sb[:, 0, d, :], in_=eTs[0, :, d, :])
                nc.scalar.dma_start(out=q_sb[:, 0, d, :], in_=qTs[0, :, d, :])
                nc.gpsimd.dma_start(out=kpe_sb[:, 0, d, :], in_=kpeTs[0, :, d, :])
            # Stripes 1-3: one big descriptor per (tensor, stripe).
            for s in range(1, KS):
                nc.sync.dma_start(out=e_sb[:, s, :, :], in_=eTs[s])
                nc.scalar.dma_start(out=q_sb[:, s, :, :], in_=qTs[s])
                nc.gpsimd.dma_start(out=kpe_sb[:, s, :, :], in_=kpeTs[s])

            for ks in range(KS):
                for qt in range(QT):
                    sq, cq = divmod(qt, KS // 1)  # qt -> (stripe, block) in q cols
                    # q columns qt*128..(qt+1)*128 live at stripe qt//4, block qt%4
                    sq, cq = qt // 4, qt % 4
                    qs = slice(cq * 128, (cq + 1) * 128)
                    pso = mps.tile([128, KSTRIPE], f32, tag="mps")
                    for d in range(DTILES):
                        nc.tensor.matmul(pso[:], q_sb[:, sq, d, qs],
                                         e_sb[:, ks, d, :],
                                         start=(d == 0), stop=False)
                    for d in range(DTILES):
                        nc.tensor.matmul(pso[:], e_sb[:, sq, d, qs],
                                         kpe_sb[:, ks, d, :],
                                         start=False, stop=(d == DTILES - 1))
                    o_t = outp.tile([128, KSTRIPE], bf16, tag="o_t")
                    if (ks * QT + qt) % 2 == 0:
                        nc.vector.tensor_copy(out=o_t[:], in_=pso[:])
                    else:
                        nc.scalar.copy(out=o_t[:], in_=pso[:])
                    nc.gpsimd.dma_start(
                        out=out16[qt * 128:(qt + 1) * 128,
                                  ks * KSTRIPE:(ks + 1) * KSTRIPE],
                        in_=o_t[:])
    nc.compile()
    return nc


def _stripe_major(x16: np.ndarray) -> np.ndarray:
    # [D, K] -> [KS, 128, DTILES, 512] with [s, p, d, c] = x[d*128+p, s*512+c]
    return np.ascontiguousarray(
        x16.reshape(DTILES, 128, KS, KSTRIPE).transpose(2, 1, 0, 3))


def kernel(q: np.ndarray, k: np.ndarray, embed: np.ndarray) -> np.ndarray:
    import ml_dtypes
    from concourse.bass_utils import run_bass_kernel_spmd

    if "nc" not in _CACHE:
        _CACHE["nc"] = _build()
    nc = _CACHE["nc"]

    bf = ml_dtypes.bfloat16
    e = np.asarray(embed[:K], dtype=np.float32)
    eTs = _stripe_major(e.T.astype(bf))
    in_maps = []
    for b in range(B):
        qTs = _stripe_major(np.asarray(q[b], dtype=np.float32).T.astype(bf))
        kpeTs = _stripe_major((np.asarray(k[b], dtype=np.float32) + e).T.astype(bf))
        in_maps.append({"qTs": qTs, "eTs": eTs, "kpeTs": kpeTs})
    res = run_bass_kernel_spmd(nc, in_maps, core_ids=list(range(B)), trace=TRACE)
    _CACHE["last_result"] = res
    return np.stack([res.results[b]["out16"].astype(np.float32) for b in range(B)])


# revision 6
# speedup vs baseline: 1.0730x; 1.0730x over previous
"""AbsoluteLearnedPE kernel v8: host-side transpose/add/bf16-cast,
device is a pure bf16 matmul stream; bf16 output upconverted on host.

Per core (data-parallel over batch): logits = q_b @ E^T + E @ (k_b+E)^T.
Host feeds stripe-major bf16 tensors [KS, 128, DTILES, 512]; all device
DMAs are [128,512] chunks in PE-consumption order.

Ring topology (measured): Sync + Scalar are fast hardware-DGE rings;
GpSimd is a slow software-DGE ring (~50GB/s). So: Sync carries the
stripe-0 eT/qT chunks then all output DMAs; Scalar carries the bulk of
the remaining input; GpSimd only carries kpe stripes 2-3 (needed last).
Evictions all on Vector (Scalar's queue is busy issuing descriptors).
6 warmup matmuls on a memset tile cover the DMA lead-in so the PE HAM
clock-gate is at 2.4GHz when real matmuls start.
"""

import numpy as np

B, Q, K, D = 8, 2048, 2048, 1024
DTILES = D // 128     # 8
QT = Q // 128         # 16
KSTRIPE = 512
KS = K // KSTRIPE     # 4
WARM_MMS = 6

_CACHE = {}
TRACE = False


def _build():
    from concourse import bacc
    import concourse.mybir as mybir
    import concourse.tile as tile

    f32 = mybir.dt.float32
    bf16 = mybir.dt.bfloat16

    nc = bacc.Bacc("TRN2", target_bir_lowering=False, debug=False, num_devices=B)
    qTs = nc.dram_tensor("qTs", [KS, 128, DTILES, KSTRIPE], bf16,
                         kind="ExternalInput").ap()
    eTs = nc.dram_tensor("eTs", [KS, 128, DTILES, KSTRIPE], bf16,
                         kind="ExternalInput").ap()
    kpeTs = nc.dram_tensor("kpeTs", [KS, 128, DTILES, KSTRIPE], bf16,
                           kind="ExternalInput").ap()
    out16 = nc.dram_tensor("out16", [Q, K], bf16, kind="ExternalOutput").ap()

    with tile.TileContext(nc) as tc:
        with tc.tile_pool(name="big", bufs=1) as big, \
             tc.tile_pool(name="outp", bufs=8) as outp, \
             tc.tile_pool(name="mps", bufs=8, space="PSUM") as mps:

            q_sb = big.tile([128, KS, DTILES, KSTRIPE], bf16, tag="qT")
            e_sb = big.tile([128, KS, DTILES, KSTRIPE], bf16, tag="eT")
            kpe_sb = big.tile([128, KS, DTILES, KSTRIPE], bf16, tag="kpeT")

            # PE warmup during the DMA lead-in.
            wtile = big.tile([128, KSTRIPE], bf16, tag="warm")
            nc.gpsimd.memset(wtile[:], 0.0)
            wps = mps.tile([128, KSTRIPE], f32, tag="mps")
            for _ in range(WARM_MMS):
                nc.tensor.matmul(wps[:], wtile[:, 0:128], wtile[:],
                                 start=True, stop=True)

            def load(eng, sb, dram, s, d):
                eng.dma_start(out=sb[:, s, d, :], in_=dram[s, :, d, :])

            # Sync ring: stripe-0 eT/qT interleaved per-d (first consumed).
            for d in range(DTILES):
                load(nc.sync, e_sb, eTs, 0, d)
                load(nc.sync, q_sb, qTs, 0, d)
            # Scalar ring: kpe s0, then (eT,qT) s1..s3 (lhsT needed early),
            # then kpe s1.
            for d in range(DTILES):
                load(nc.scalar, kpe_sb, kpeTs, 0, d)
            for s in range(1, KS):
                for d in range(DTILES):
                    load(nc.scalar, e_sb, eTs, s, d)
                    load(nc.scalar, q_sb, qTs, s, d)
            for d in range(DTILES):
                load(nc.scalar, kpe_sb, kpeTs, 1, d)
            # GpSimd (slow software ring): kpe s2/s3, needed 110us+ in.
            for s in (2, 3):
                for d in range(DTILES):
                    load(nc.gpsimd, kpe_sb, kpeTs, s, d)

            for ks in range(KS):
                for qt in range(QT):
                    sq, cq = qt // 4, qt % 4
                    qs = slice(cq * 128, (cq + 1) * 128)
                    pso = mps.tile([128, KSTRIPE], f32, tag="mps")
                    for d in range(DTILES):
                        nc.tensor.matmul(pso[:], q_sb[:, sq, d, qs],
                                         e_sb[:, ks, d, :],
                                         start=(d == 0), stop=False)
                    for d in range(DTILES):
                        nc.tensor.matmul(pso[:], e_sb[:, sq, d, qs],
                                         kpe_sb[:, ks, d, :],
                                         start=False, stop=(d == DTILES - 1))
                    o_t = outp.tile([128, KSTRIPE], bf16, tag="o_t")
                    nc.vector.tensor_copy(out=o_t[:], in_=pso[:])
                    nc.sync.dma_start(
                        out=out16[qt * 128:(qt + 1) * 128,
                                  ks * KSTRIPE:(ks + 1) * KSTRIPE],
                        in_=o_t[:])
    nc.compile()
    return nc


def _stripe_major(x16: np.ndarray) -> np.ndarray:
    # [D, K] -> [KS, 128, DTILES, 512] with [s, p, d, c] = x[d*128+p, s*512+c]
    return np.ascontiguousarray(
        x16.reshape(DTILES, 128, KS, KSTRIPE).transpose(2, 1, 0, 3))


def kernel(q: np.ndarray, k: np.ndarray, embed: np.ndarray) -> np.ndarray:
    import ml_dtypes
    from concourse.bass_utils import run_bass_kernel_spmd

    if "nc" not in _CACHE:
        _CACHE["nc"] = _build()
    nc = _CACHE["nc"]

    bf = ml_dtypes.bfloat16
    e = np.asarray(embed[:K], dtype=np.float32)
    eTs = _stripe_major(e.T.astype(bf))
    in_maps = []
    for b in range(B):
        qTs = _stripe_major(np.asarray(q[b], dtype=np.float32).T.astype(bf))
        kpeTs = _stripe_major((np.asarray(k[b], dtype=np.float32) + e).T.astype(bf))
        in_maps.append({"qTs": qTs, "eTs": eTs, "kpeTs": kpeTs})
    res = run_bass_kernel_spmd(nc, in_maps, core_ids=list(range(B)), trace=TRACE)
    _CACHE["last_result"] = res
    return np.stack([res.results[b]["out16"].astype(np.float32) for b in range(B)])
